# revision 9
# baseline (speedup 1.0000x reference)
"""DeepSeek MoE gate (sigmoid routing, grouped top-k) for 8x Trainium2 NeuronCores.

Strategy: data-parallel over tokens (16384 tokens -> 2048 per core). The
gate weight is pre-split (bf16 hi/lo) and pre-transposed on the HOST into
whl[p, j, 0:256] = WhT chunk j, whl[:, j, 256:512] = WlT chunk j, then
replicated to all cores - no PE cycles are spent on W at all. Per core,
for each 128-token x tile [128, 7168] fp32:
  - stream halves on the two HWDGE queues,
  - PE fp32-transposes (4-chunk PSUM units) + bf16 hi/lo split on the way
    to SBUF (hi = ACT cast, lo = DVE residual subtract),
  - gate matmuls per chunk j accumulate one PSUM bank [128, 512]:
    xh @ [Wh|Wl] (N=512) and xl @ Wh (N=256), so
    logits = xh@Wh + xh@Wl + xl@Wh (the O(2^-18) xl@Wl term is dropped),
  - logits = A[:, :256] + A[:, 256:] (ACT stage + DVE add), sigmoid on
    ACT, +bias, grouped max, native top-8 (InstMax/InstMaxIndex),
    normalize on DVE; outputs stream out in 4-tile groups.
A block of data-independent identity self-transposes at t=0 warms the PE
clock (HAM) before x0 arrives, so the real transposes run at full rate.
"""

import os
import sys

sys.path.insert(0, "/opt/trn_rl_repo")

import numpy as np
import ml_dtypes

import concourse.bass as bass
import concourse.mybir as mybir
import concourse.tile as tile
from concourse.bass_utils import run_bass_kernel_spmd
from concourse.masks import make_identity

P = 128
H = 7168
E = 256
G = 8  # n_group
GSZ = E // G
TOPK_G = 4
TOPK = 8
N_CORES = 8
T_FULL = 4 * 4096
T_CORE = T_FULL // N_CORES
HC = H // P  # 56 contraction chunks

F32 = mybir.dt.float32
BF16 = mybir.dt.bfloat16
U32 = mybir.dt.uint32

H2 = H // 2
HC2 = HC // 2

TCH = 4  # transpose chunks per PSUM unit (exactly 1 bank)
PE_UNITS = HC // TCH  # 14
# j-positions in the matmul stream where next-tile transpose units go
TR_SLOTS = list(range(18, 56, max(1, 38 // PE_UNITS)))[:PE_UNITS]
WARMUP_T = 56  # data-independent PE transposes to release the HAM throttle


def build_moe_gate(tc: tile.TileContext, x_d, whl_d, b_d, wout_d, iout_d,
                   t_core, ctx=None):
    nc = tc.nc
    nt = t_core // P

    const_pool = ctx.enter_context(tc.tile_pool(name="const", bufs=1))
    xin_pool = ctx.enter_context(tc.tile_pool(name="xin", bufs=2))
    xhl_pool = ctx.enter_context(tc.tile_pool(name="xhl", bufs=2))
    ps_t_pool = ctx.enter_context(tc.tile_pool(name="ps_t", bufs=3, space="PSUM"))
    ps_l_pool = ctx.enter_context(tc.tile_pool(name="ps_l", bufs=2, space="PSUM"))
    sc_pool = ctx.enter_context(tc.tile_pool(name="scores", bufs=2))
    sm_pool = ctx.enter_context(tc.tile_pool(name="small", bufs=4))
    out_pool = ctx.enter_context(tc.tile_pool(name="outs", bufs=1))

    identity = const_pool.tile([P, P], F32)
    make_identity(nc, identity)

    # warm the PE clock with dummy self-transposes (no data dependencies)
    for w0 in range(0, WARMUP_T, TCH):
        pt = ps_t_pool.tile([P, TCH, P], F32, tag="ps_t")
        for q in range(TCH):
            nc.tensor.matmul(pt[:, q, :], identity, identity,
                             is_transpose=True,
                             start=(q == 0), stop=(q == TCH - 1))

    whl = const_pool.tile([P, HC, 2 * E], BF16)
    # W (host-pretransposed, bf16 hi|lo) in two halves on the ACT queue
    nc.scalar.dma_start(whl[:, :HC2, :], whl_d[:, :HC2, :])
    nc.scalar.dma_start(whl[:, HC2:, :], whl_d[:, HC2:, :])

    bias_rep = const_pool.tile([P, E], F32)
    nc.sync.dma_start(bias_rep, b_d[None, :].to_broadcast([P, E]))

    def load_x(i):
        """x tile halves, one per HWDGE queue."""
        x_sb = xin_pool.tile([P, H], F32, tag="xin")
        nc.sync.dma_start(x_sb[:, :H2], x_d[i * P:(i + 1) * P, :H2])
        nc.scalar.dma_start(x_sb[:, H2:], x_d[i * P:(i + 1) * P, H2:])
        return x_sb

    def emit_tr_unit(x_sb, xh, xl, u):
        """One PE transpose unit (4 chunks): PE fp32 transposes into one
        PSUM bank + ACT hi-cast + DVE lo-residual to SBUF bf16."""
        j0 = u * TCH
        pt = ps_t_pool.tile([P, TCH, P], F32, tag="ps_t")
        for q in range(TCH):
            j = j0 + q
            nc.tensor.matmul(pt[:, q, :], x_sb[:, j * P:(j + 1) * P],
                             identity, is_transpose=True,
                             start=(q == 0), stop=(q == TCH - 1))
        hi = xh[:, j0:j0 + TCH, :]
        nc.scalar.copy(hi, pt)
        nc.vector.tensor_sub(xl[:, j0:j0 + TCH, :], pt, hi)

    # ---- startup: x0/x1 in flight, transpose x0 ----
    x0_sb = load_x(0)
    x1_sb = load_x(1) if nt > 1 else None
    xh0 = xhl_pool.tile([P, HC, P], BF16, tag="xh")
    xl0 = xhl_pool.tile([P, HC, P], BF16, tag="xl")
    for u in range(PE_UNITS):
        emit_tr_unit(x0_sb, xh0, xl0, u)
    xhl_prefetch = {0: (xh0, xl0)}

    wout_sb = out_pool.tile([P, nt, TOPK], F32)
    iout_sb = out_pool.tile([P, nt, TOPK], U32)

    # ---- main loop ----
    for i in range(nt):
        xh, xl = xhl_prefetch.pop(i)
        nxt_x = x1_sb if i == 0 else (load_x(i + 1) if i + 1 < nt else None)
        if nxt_x is not None:
            nxh = xhl_pool.tile([P, HC, P], BF16, tag="xh")
            nxl = xhl_pool.tile([P, HC, P], BF16, tag="xl")
            xhl_prefetch[i + 1] = (nxh, nxl)

        lg = ps_l_pool.tile([P, 2 * E], F32, tag="ps_l")
        u_next = 0
        for j in range(HC):
            if (nxt_x is not None and u_next < PE_UNITS
                    and u_next < len(TR_SLOTS) and j == TR_SLOTS[u_next]):
                emit_tr_unit(nxt_x, nxh, nxl, u_next)
                u_next += 1
            nc.tensor.matmul(lg, xh[:, j, :], whl[:, j, :],
                             start=(j == 0), stop=False)
            nc.tensor.matmul(lg[:, :E], xl[:, j, :], whl[:, j, :E],
                             start=False, stop=(j == HC - 1))
        while nxt_x is not None and u_next < PE_UNITS:
            emit_tr_unit(nxt_x, nxh, nxl, u_next)
            u_next += 1

        # logits = A[:, :256] + A[:, 256:512]; scores = sigmoid(logits)+bias
        lg_hi = sc_pool.tile([P, E], F32, tag="lg_hi")
        nc.scalar.copy(lg_hi, lg[:, E:])
        logits = sc_pool.tile([P, E], F32, tag="logits")
        nc.vector.tensor_add(logits, lg[:, :E], lg_hi)
        scores = sc_pool.tile([P, E], F32, tag="scores")
        nc.scalar.activation(scores, logits,
                             mybir.ActivationFunctionType.Sigmoid)
        nc.gpsimd.tensor_add(scores, scores, bias_rep)

        scores_g = scores.rearrange("p (g e) -> p g e", g=G)
        gmax = sm_pool.tile([P, G], F32, tag="gmax")
        nc.vector.reduce_max(gmax, scores_g, axis=mybir.AxisListType.X)

        g8 = sm_pool.tile([P, 8], F32, tag="g8")
        nc.vector.max(out=g8, in_=gmax)

        gmask = sm_pool.tile([P, G], F32, tag="gmask")
        nc.vector.tensor_scalar(gmask, gmax, g8[:, TOPK_G - 1:TOPK_G], None,
                                op0=mybir.AluOpType.is_ge)

        masked = sc_pool.tile([P, E], F32, tag="masked")
        nc.gpsimd.tensor_tensor(
            masked.rearrange("p (g e) -> p g e", g=G), scores_g,
            gmask[:, :, None].to_broadcast([P, G, GSZ]),
            op=mybir.AluOpType.mult)

        m8 = sm_pool.tile([P, 8], F32, tag="m8")
        nc.vector.max(out=m8, in_=masked)
        nc.vector.max_index(iout_sb[:, i, :], m8, masked)

        ssum = sm_pool.tile([P, 1], F32, tag="ssum")
        nc.vector.reduce_sum(ssum, m8, axis=mybir.AxisListType.X)
        nc.vector.tensor_scalar_add(ssum, ssum, 1e-6)
        rcp = sm_pool.tile([P, 1], F32, tag="rcp")
        nc.vector.reciprocal(rcp, ssum)
        nc.vector.tensor_scalar_mul(wout_sb[:, i, :], m8, rcp)

        # stream outputs in 4-tile groups
        if i % 4 == 3:
            g0 = i - 3
            nc.sync.dma_start(wout_d[:, g0:i + 1, :], wout_sb[:, g0:i + 1, :])
            nc.scalar.dma_start(iout_d[:, g0:i + 1, :], iout_sb[:, g0:i + 1, :])


def build_bass(t_core=T_CORE):
    from concourse import bacc
    nc = bacc.Bacc("TRN2", target_bir_lowering=False, debug=False,
                   num_devices=N_CORES)
    nt = t_core // P
    x_d = nc.dram_tensor("x", [t_core, H], F32, kind="ExternalInput").ap()
    whl_d = nc.dram_tensor("whl", [P, HC, 2 * E], BF16,
                           kind="ExternalInput").ap()
    b_d = nc.dram_tensor("b", [E], F32, kind="ExternalInput").ap()
    wout_d = nc.dram_tensor("wout", [P, nt, TOPK], F32,
                            kind="ExternalOutput").ap()
    iout_d = nc.dram_tensor("iout", [P, nt, TOPK], U32,
                            kind="ExternalOutput").ap()
    from contextlib import ExitStack
    with tile.TileContext(nc) as tc:
        with ExitStack() as ctx:
            build_moe_gate(tc, x_d, whl_d, b_d, wout_d, iout_d, t_core,
                           ctx=ctx)
    nc.compile()
    return nc


_NC_CACHE = {}


def _get_nc():
    key = "main"
    if key not in _NC_CACHE:
        _NC_CACHE[key] = build_bass()
    return _NC_CACHE[key]


def _pack_whl(w):
    """Host-side W prep: bf16 hi/lo split + transpose into
    whl[p, j, 0:256] = Wh.T chunk j, whl[p, j, 256:512] = Wl.T chunk j."""
    w = np.asarray(w, dtype=np.float32)
    wh = w.astype(ml_dtypes.bfloat16)
    wl = (w - wh.astype(np.float32)).astype(ml_dtypes.bfloat16)
    # [E, H] -> [H, E] -> [HC, P, E] -> [P, HC, E]
    whT = np.ascontiguousarray(wh.T.reshape(HC, P, E).transpose(1, 0, 2))
    wlT = np.ascontiguousarray(wl.T.reshape(HC, P, E).transpose(1, 0, 2))
    return np.ascontiguousarray(np.concatenate([whT, wlT], axis=2))


def kernel(hidden_states, gate_weight, bias, n_group, topk_group, top_k,
           _trace=False):
    assert int(n_group) == G and int(topk_group) == TOPK_G and int(top_k) == TOPK
    x = np.asarray(hidden_states, dtype=np.float32)
    whl = _pack_whl(gate_weight)
    b = np.ascontiguousarray(np.asarray(bias, dtype=np.float32))
    B, S, _ = x.shape
    xf = x.reshape(-1, H)
    assert xf.shape[0] == T_FULL

    nc = _get_nc()
    in_maps = []
    for c in range(N_CORES):
        in_maps.append({
            "x": np.ascontiguousarray(xf[c * T_CORE:(c + 1) * T_CORE]),
            "whl": whl,
            "b": b,
        })
    try:
        res = run_bass_kernel_spmd(nc, in_maps, core_ids=list(range(N_CORES)),
                                   trace=_trace)
    except ModuleNotFoundError:
        res = run_bass_kernel_spmd(nc, in_maps, core_ids=list(range(N_CORES)),
                                   trace=False)
    weights = np.empty((T_FULL, TOPK), dtype=np.float32)
    indices = np.empty((T_FULL, TOPK), dtype=np.int32)
    for c, r in enumerate(res.results):
        wc = np.transpose(r["wout"], (1, 0, 2)).reshape(T_CORE, TOPK)
        ic = np.transpose(r["iout"], (1, 0, 2)).reshape(T_CORE, TOPK)
        weights[c * T_CORE:(c + 1) * T_CORE] = wc
        indices[c * T_CORE:(c + 1) * T_CORE] = ic.astype(np.int32)
    out_w = weights.reshape(B, S, TOPK)
    out_i = indices.reshape(B, S, TOPK)
    if _trace:
        return (out_w, out_i), res
    return out_w, out_i


# revision 13
# speedup vs baseline: 1.1817x; 1.1817x over previous
"""DeepSeek MoE gate (sigmoid routing, grouped top-k) for 8x Trainium2 NeuronCores.

Strategy: data-parallel over tokens (16384 tokens -> 2048 per core). The
gate weight is pre-split (bf16 hi/lo) and pre-transposed on the HOST into
whl[p, j, 0:256] = WhT chunk j, whl[:, j, 256:512] = WlT chunk j, then
replicated to all cores - no PE cycles are spent on W at all. Per core,
for each 128-token x tile [128, 7168] fp32:
  - stream halves on the two HWDGE queues,
  - PE fp32-transposes (4-chunk PSUM units) + bf16 hi/lo split on the way
    to SBUF (hi = ACT cast, lo = DVE residual subtract),
  - gate matmuls per chunk j accumulate one PSUM bank [128, 512]:
    xh @ [Wh|Wl] (N=512) and xl @ Wh (N=256), so
    logits = xh@Wh + xh@Wl + xl@Wh (the O(2^-18) xl@Wl term is dropped),
  - logits = A[:, :256] + A[:, 256:] (ACT stage + DVE add), sigmoid on
    ACT, +bias, grouped max, native top-8 (InstMax/InstMaxIndex),
    normalize on DVE; outputs stream out in 4-tile groups.
A block of data-independent identity self-transposes at t=0 warms the PE
clock (HAM) before x0 arrives, so the real transposes run at full rate.
"""

import os
import sys

sys.path.insert(0, "/opt/trn_rl_repo")

import numpy as np
import ml_dtypes

import concourse.bass as bass
import concourse.mybir as mybir
import concourse.tile as tile
from concourse.bass_utils import run_bass_kernel_spmd
from concourse.masks import make_identity

P = 128
H = 7168
E = 256
G = 8  # n_group
GSZ = E // G
TOPK_G = 4
TOPK = 8
N_CORES = 8
T_FULL = 4 * 4096
T_CORE = T_FULL // N_CORES
HC = H // P  # 56 contraction chunks

F32 = mybir.dt.float32
BF16 = mybir.dt.bfloat16
U32 = mybir.dt.uint32

H2 = H // 2
HC2 = HC // 2

TCH = 7  # transpose chunks per PSUM unit
PE_UNITS = HC // TCH  # 8
WARMUP_T = 56  # data-independent PE transposes to release the HAM throttle


def build_moe_gate(tc: tile.TileContext, x_d, whl_d, b_d, wout_d, iout_d,
                   t_core, ctx=None):
    nc = tc.nc
    nt = t_core // P

    const_pool = ctx.enter_context(tc.tile_pool(name="const", bufs=1))
    xin_pool = ctx.enter_context(tc.tile_pool(name="xin", bufs=2))
    xhl_pool = ctx.enter_context(tc.tile_pool(name="xhl", bufs=2))
    ps_t_pool = ctx.enter_context(tc.tile_pool(name="ps_t", bufs=3, space="PSUM"))
    ps_l_pool = ctx.enter_context(tc.tile_pool(name="ps_l", bufs=2, space="PSUM"))
    sc_pool = ctx.enter_context(tc.tile_pool(name="scores", bufs=2))
    sm_pool = ctx.enter_context(tc.tile_pool(name="small", bufs=4))
    out_pool = ctx.enter_context(tc.tile_pool(name="outs", bufs=1))

    identity = const_pool.tile([P, P], F32)
    make_identity(nc, identity)

    # warm the PE clock with dummy self-transposes (no data dependencies)
    for w0 in range(0, WARMUP_T, TCH):
        pt = ps_t_pool.tile([P, TCH, P], F32, tag="ps_t")
        for q in range(TCH):
            nc.tensor.matmul(pt[:, q, :], identity, identity,
                             is_transpose=True,
                             start=(q % 4 == 0),
                             stop=(q % 4 == 3 or q == TCH - 1))

    whl = const_pool.tile([P, HC, 2 * E], BF16)
    # W (host-pretransposed, bf16 hi|lo) in two halves on the ACT queue
    nc.scalar.dma_start(whl[:, :HC2, :], whl_d[:, :HC2, :])
    nc.scalar.dma_start(whl[:, HC2:, :], whl_d[:, HC2:, :])

    bias_rep = const_pool.tile([P, E], F32)
    nc.sync.dma_start(bias_rep, b_d[None, :].to_broadcast([P, E]))

    def load_x(i):
        """x tile halves, one per HWDGE queue."""
        x_sb = xin_pool.tile([P, H], F32, tag="xin")
        nc.sync.dma_start(x_sb[:, :H2], x_d[i * P:(i + 1) * P, :H2])
        nc.scalar.dma_start(x_sb[:, H2:], x_d[i * P:(i + 1) * P, H2:])
        return x_sb

    def emit_tr_unit(x_sb, xh, xl, u):
        """One PE transpose unit (4 chunks): PE fp32 transposes into one
        PSUM bank + ACT hi-cast + DVE lo-residual to SBUF bf16."""
        j0 = u * TCH
        pt = ps_t_pool.tile([P, TCH, P], F32, tag="ps_t")
        for q in range(TCH):
            j = j0 + q
            nc.tensor.matmul(pt[:, q, :], x_sb[:, j * P:(j + 1) * P],
                             identity, is_transpose=True,
                             start=(q % 4 == 0),
                             stop=(q % 4 == 3 or q == TCH - 1))
        hi = xh[:, j0:j0 + TCH, :]
        nc.scalar.copy(hi, pt)
        nc.vector.tensor_sub(xl[:, j0:j0 + TCH, :], pt, hi)

    # ---- startup: x0/x1 in flight, transpose x0 ----
    x0_sb = load_x(0)
    x1_sb = load_x(1) if nt > 1 else None
    xh0 = xhl_pool.tile([P, HC, P], BF16, tag="xh")
    xl0 = xhl_pool.tile([P, HC, P], BF16, tag="xl")
    for u in range(PE_UNITS):
        emit_tr_unit(x0_sb, xh0, xl0, u)
    xhl_prefetch = {0: (xh0, xl0)}

    wout_sb = out_pool.tile([P, nt, TOPK], F32)
    iout_sb = out_pool.tile([P, nt, TOPK], U32)

    # ---- main loop ----
    for i in range(nt):
        xh, xl = xhl_prefetch.pop(i)
        nxt_x = x1_sb if i == 0 else (load_x(i + 1) if i + 1 < nt else None)
        if nxt_x is not None:
            nxh = xhl_pool.tile([P, HC, P], BF16, tag="xh")
            nxl = xhl_pool.tile([P, HC, P], BF16, tag="xl")
            xhl_prefetch[i + 1] = (nxh, nxl)

        lg = ps_l_pool.tile([P, 2 * E], F32, tag="ps_l")
        for u in range(PE_UNITS):
            if nxt_x is not None:
                emit_tr_unit(nxt_x, nxh, nxl, u)
            for j in range(u * TCH, (u + 1) * TCH):
                nc.tensor.matmul(lg, xh[:, j, :], whl[:, j, :],
                                 start=(j == 0), stop=False)
                nc.tensor.matmul(lg[:, :E], xl[:, j, :], whl[:, j, :E],
                                 start=False, stop=(j == HC - 1))

        # logits = A[:, :256] + A[:, 256:512]; scores = sigmoid(logits)+bias
        lg_hi = sc_pool.tile([P, E], F32, tag="lg_hi")
        nc.scalar.copy(lg_hi, lg[:, E:])
        logits = sc_pool.tile([P, E], F32, tag="logits")
        nc.vector.tensor_add(logits, lg[:, :E], lg_hi)
        scores = sc_pool.tile([P, E], F32, tag="scores")
        nc.scalar.activation(scores, logits,
                             mybir.ActivationFunctionType.Sigmoid)
        nc.gpsimd.tensor_add(scores, scores, bias_rep)

        scores_g = scores.rearrange("p (g e) -> p g e", g=G)
        gmax = sm_pool.tile([P, G], F32, tag="gmax")
        nc.vector.reduce_max(gmax, scores_g, axis=mybir.AxisListType.X)

        g8 = sm_pool.tile([P, 8], F32, tag="g8")
        nc.vector.max(out=g8, in_=gmax)

        gmask = sm_pool.tile([P, G], F32, tag="gmask")
        nc.vector.tensor_scalar(gmask, gmax, g8[:, TOPK_G - 1:TOPK_G], None,
                                op0=mybir.AluOpType.is_ge)

        masked = sc_pool.tile([P, E], F32, tag="masked")
        nc.gpsimd.tensor_tensor(
            masked.rearrange("p (g e) -> p g e", g=G), scores_g,
            gmask[:, :, None].to_broadcast([P, G, GSZ]),
            op=mybir.AluOpType.mult)

        m8 = sm_pool.tile([P, 8], F32, tag="m8")
        nc.vector.max(out=m8, in_=masked)
        nc.vector.max_index(iout_sb[:, i, :], m8, masked)

        ssum = sm_pool.tile([P, 1], F32, tag="ssum")
        nc.vector.reduce_sum(ssum, m8, axis=mybir.AxisListType.X)
        nc.vector.tensor_scalar_add(ssum, ssum, 1e-6)
        rcp = sm_pool.tile([P, 1], F32, tag="rcp")
        nc.vector.reciprocal(rcp, ssum)
        nc.vector.tensor_scalar_mul(wout_sb[:, i, :], m8, rcp)

        # stream outputs in 4-tile groups
        if i % 4 == 3:
            g0 = i - 3
            nc.sync.dma_start(wout_d[:, g0:i + 1, :], wout_sb[:, g0:i + 1, :])
            nc.scalar.dma_start(iout_d[:, g0:i + 1, :], iout_sb[:, g0:i + 1, :])


def build_bass(t_core=T_CORE):
    from concourse import bacc
    nc = bacc.Bacc("TRN2", target_bir_lowering=False, debug=False,
                   num_devices=N_CORES)
    nt = t_core // P
    x_d = nc.dram_tensor("x", [t_core, H], F32, kind="ExternalInput").ap()
    whl_d = nc.dram_tensor("whl", [P, HC, 2 * E], BF16,
                           kind="ExternalInput").ap()
    b_d = nc.dram_tensor("b", [E], F32, kind="ExternalInput").ap()
    wout_d = nc.dram_tensor("wout", [P, nt, TOPK], F32,
                            kind="ExternalOutput").ap()
    iout_d = nc.dram_tensor("iout", [P, nt, TOPK], U32,
                            kind="ExternalOutput").ap()
    from contextlib import ExitStack
    with tile.TileContext(nc) as tc:
        with ExitStack() as ctx:
            build_moe_gate(tc, x_d, whl_d, b_d, wout_d, iout_d, t_core,
                           ctx=ctx)
    nc.compile()
    return nc


_NC_CACHE = {}


def _get_nc():
    key = "main"
    if key not in _NC_CACHE:
        _NC_CACHE[key] = build_bass()
    return _NC_CACHE[key]


def _pack_whl(w):
    """Host-side W prep: bf16 hi/lo split + transpose into
    whl[p, j, 0:256] = Wh.T chunk j, whl[p, j, 256:512] = Wl.T chunk j."""
    w = np.asarray(w, dtype=np.float32)
    wh = w.astype(ml_dtypes.bfloat16)
    wl = (w - wh.astype(np.float32)).astype(ml_dtypes.bfloat16)
    # [E, H] -> [H, E] -> [HC, P, E] -> [P, HC, E]
    whT = np.ascontiguousarray(wh.T.reshape(HC, P, E).transpose(1, 0, 2))
    wlT = np.ascontiguousarray(wl.T.reshape(HC, P, E).transpose(1, 0, 2))
    return np.ascontiguousarray(np.concatenate([whT, wlT], axis=2))


def kernel(hidden_states, gate_weight, bias, n_group, topk_group, top_k,
           _trace=False):
    assert int(n_group) == G and int(topk_group) == TOPK_G and int(top_k) == TOPK
    x = np.asarray(hidden_states, dtype=np.float32)
    whl = _pack_whl(gate_weight)
    b = np.ascontiguousarray(np.asarray(bias, dtype=np.float32))
    B, S, _ = x.shape
    xf = x.reshape(-1, H)
    assert xf.shape[0] == T_FULL

    nc = _get_nc()
    in_maps = []
    for c in range(N_CORES):
        in_maps.append({
            "x": np.ascontiguousarray(xf[c * T_CORE:(c + 1) * T_CORE]),
            "whl": whl,
            "b": b,
        })
    try:
        res = run_bass_kernel_spmd(nc, in_maps, core_ids=list(range(N_CORES)),
                                   trace=_trace)
    except ModuleNotFoundError:
        res = run_bass_kernel_spmd(nc, in_maps, core_ids=list(range(N_CORES)),
                                   trace=False)
    weights = np.empty((T_FULL, TOPK), dtype=np.float32)
    indices = np.empty((T_FULL, TOPK), dtype=np.int32)
    for c, r in enumerate(res.results):
        wc = np.transpose(r["wout"], (1, 0, 2)).reshape(T_CORE, TOPK)
        ic = np.transpose(r["iout"], (1, 0, 2)).reshape(T_CORE, TOPK)
        weights[c * T_CORE:(c + 1) * T_CORE] = wc
        indices[c * T_CORE:(c + 1) * T_CORE] = ic.astype(np.int32)
    out_w = weights.reshape(B, S, TOPK)
    out_i = indices.reshape(B, S, TOPK)
    if _trace:
        return (out_w, out_i), res
    return out_w, out_i
